# revision 16
# baseline (speedup 1.0000x reference)
"""Multi-modal causal cross-attention + MLP on 8 Trainium2 NeuronCores.

Problem (hardcoded): B=4, S=2048, C=1024, H=16, HS=64, M=3.
    q = einsum('bsc,hcd->bhsd', query_x, Wq)
    per modality m: kv = einsum('bsc,hcd->bhsd', kv_x[m], Wkv[m]); k, v = split
    out += causal-softmax(q k^T / sqrt(hs)) @ v     (summed over m)
    y = tanh(concat-heads(out) @ W1 + b1) @ W2 + b2

Sharding: data-parallel over batch (4) x tensor-parallel over heads (2 groups
of 8), one (batch, head-group) per core.  Host adds the two head-group y1
partials per batch; launch B does tanh + W2 on per-core q-halves.

v3 (launch A, per core):
  - inputs HOST-pretransposed (xqT [C,S], xkvT [M,C,S]).
  - prologue: q-projection + modality-0 kv-projection, dense full-K matmuls.
  - attention loops m -> qt -> hp; modality m+1's kv-projection matmuls are
    PACED into modality m's attention stream (1 per score-block + a burst of
    8 at each head-pair boundary) so the PE never idles while ScalarE runs
    the exps; this also keeps HAM at K=8/8.
  - scores for a head pair are back-to-back row-tiled matmuls (rows 0-63 /
    64-127); one exp covers both heads ([128,2,512] psum pair).
  - W1 partial per (qt, m) right after the qt's attention, accumulated into
    a bf16 SBUF y1acc (frees PSUM, spreads W1 into the stream).
  - softmax denominators: vector copy of psum row 64, in-place fast
    reciprocal, gpsimd partition-broadcast, two multiplies.
"""

import numpy as np
import ml_dtypes

import concourse.bass as bass
import concourse.tile as tile
from concourse import bacc, mybir
from concourse.bass_utils import run_bass_kernel_spmd

BF = ml_dtypes.bfloat16
F32 = mybir.dt.float32
BF16 = mybir.dt.bfloat16

B, S, C, H, HS, M3 = 4, 2048, 1024, 16, 64, 3
N_CORES = 8
EXP = mybir.ActivationFunctionType.Exp
TANH = mybir.ActivationFunctionType.Tanh
MULT = mybir.AluOpType.mult
ADD = mybir.AluOpType.add

_CACHE = {}


class Feeder:
    """Queue of deferred instruction-issuing callables, paced into the
    attention stream to keep the PE dense while ScalarE runs exps.
    Steps are tagged with chain-end markers so a shared PSUM bank is never
    handed to another chain while a projection chain is half-issued."""

    def __init__(self):
        self.steps = []
        self.at_boundary = True

    def add(self, fn, ends_chain):
        self.steps.append((fn, ends_chain))

    def feed(self, n):
        while n > 0 and self.steps:
            fn, ends = self.steps.pop(0)
            fn()
            self.at_boundary = ends
            n -= 1

    def finish_chain(self):
        while self.steps and not self.at_boundary:
            fn, ends = self.steps.pop(0)
            fn()
            self.at_boundary = ends

    def drain(self):
        self.feed(len(self.steps))


class MultiFeeder:
    """FIFO over several Feeders (kv0 tail, kv1, kv2, ...)."""

    def __init__(self):
        self.fs = []
        self.fed = 0

    def append(self, f):
        self.fs.append(f)

    def _cur(self):
        while self.fs and not self.fs[0].steps:
            self.fs.pop(0)
        return self.fs[0] if self.fs else None

    def feed(self, n):
        while n > 0:
            f = self._cur()
            if f is None:
                return
            take = min(n, len(f.steps))
            f.feed(take)
            self.fed += take
            n -= take

    def finish_chain(self):
        f = self._cur()
        if f is not None:
            f.finish_chain()


def _build_launch_a():
    nc = bacc.Bacc("TRN2", target_bir_lowering=False, debug=False, num_devices=N_CORES)
    xqT = nc.dram_tensor("xqT", [C, S], BF16, kind="ExternalInput").ap()
    xkvT = nc.dram_tensor("xkvT", [M3, C, S], BF16, kind="ExternalInput").ap()
    wq = nc.dram_tensor("wq", [8, 128, 512], BF16, kind="ExternalInput").ap()
    wk = nc.dram_tensor("wk", [M3, 8, 128, 512], BF16, kind="ExternalInput").ap()
    wv = nc.dram_tensor("wv", [M3, 8, 128, 512], BF16, kind="ExternalInput").ap()
    w1 = nc.dram_tensor("w1", [128, 4, 4, 128], BF16, kind="ExternalInput").ap()
    tri = nc.dram_tensor("tri", [128, 128], BF16, kind="ExternalInput").ap()
    y1p = nc.dram_tensor("y1p", [4, 128, S], BF16, kind="ExternalOutput").ap()

    with tile.TileContext(nc) as tc:
        import contextlib
        with contextlib.ExitStack() as stk:
            singles = stk.enter_context(tc.tile_pool(name="singles", bufs=1))
            qT_sb = singles.tile([128, 4, S], BF16, tag="qT")
            w1_sb = singles.tile([128, 4, 4, 128], BF16, tag="w1")
            tri_sb = singles.tile([128, 128], BF16, tag="tri")
            y1acc = singles.tile([128, 4, 4, 512], BF16, tag="y1acc")

            kTp = stk.enter_context(tc.tile_pool(name="kTp", bufs=2))
            vEp = stk.enter_context(tc.tile_pool(name="vEp", bufs=2))
            wtsp = stk.enter_context(tc.tile_pool(name="wts", bufs=2))
            xts = stk.enter_context(tc.tile_pool(name="xts", bufs=16))
            probs_p = stk.enter_context(tc.tile_pool(name="probs", bufs=4))
            dnm_p = stk.enter_context(tc.tile_pool(name="dnm", bufs=2))
            rbc_p = stk.enter_context(tc.tile_pool(name="rbc", bufs=2))
            av_p = stk.enter_context(tc.tile_pool(name="avp", bufs=2))
            stg_p = stk.enter_context(tc.tile_pool(name="stgp", bufs=2))

            nc.sync.dma_start(out=tri_sb[:], in_=tri[:])
            nc.sync.dma_start(out=w1_sb[:], in_=w1[:])

            kT_t = [None] * M3
            vE_t = [None] * M3

            def new_kv_tiles(m):
                kT_t[m] = kTp.tile([128, 4, S], BF16, tag="kT", name=f"kT{m}")
                vE_t[m] = vEp.tile([128, 16, 8, 65], BF16, tag="vE", name=f"vE{m}")
                nc.vector.memset(vE_t[m][:, :, :, 64:65], 1.0)

            def load_kv_weights(m):
                wk_sb = wtsp.tile([128, 8, 512], BF16, tag="wk", name=f"wk{m}")
                wv_sb = wtsp.tile([128, 8, 512], BF16, tag="wv", name=f"wv{m}")
                nc.sync.dma_start(out=wk_sb[:], in_=wk[m].rearrange("c p j -> p c j"))
                nc.sync.dma_start(out=wv_sb[:], in_=wv[m].rearrange("c p j -> p c j"))
                return wk_sb, wv_sb

            # ---------------- prologue: q proj + modality-0 kv proj ----------
            with tc.tile_pool(name="pp", bufs=8, space="PSUM") as pp:
                wq_sb = wtsp.tile([128, 8, 512], BF16, tag="wk", name="wq")
                nc.sync.dma_start(out=wq_sb[:], in_=wq.rearrange("c p j -> p c j"))
                for sb in range(4):
                    ps = [pp.tile([128, 512], F32, tag="acc", name=f"psq{i}") for i in range(4)]
                    for c in range(8):
                        xt = xts.tile([128, 512], BF16, tag="xt")
                        nc.sync.dma_start(
                            out=xt[:],
                            in_=xqT[c * 128:(c + 1) * 128, sb * 512:(sb + 1) * 512],
                        )
                        for p in range(4):
                            nc.tensor.matmul(
                                ps[p][:], wq_sb[:, c, p * 128:(p + 1) * 128], xt[:],
                                start=(c == 0), stop=(c == 7),
                            )
                    for p in range(4):
                        nc.vector.tensor_copy(qT_sb[:, p, sb * 512:(sb + 1) * 512], ps[p][:])

                new_kv_tiles(0)
                kv0_w = load_kv_weights(0)
                wk_sb, wv_sb = kv0_w
                for sb in range(1):
                    psk = [pp.tile([128, 512], F32, tag="acc", name=f"psk{i}") for i in range(4)]
                    psv = [pp.tile([128, 8, 64], F32, tag="acc", name=f"psv{i}") for i in range(4)]
                    for c in range(8):
                        xt = xts.tile([128, 512], BF16, tag="xt")
                        nc.sync.dma_start(
                            out=xt[:],
                            in_=xkvT[0, c * 128:(c + 1) * 128, sb * 512:(sb + 1) * 512],
                        )
                        for p in range(4):
                            nc.tensor.matmul(
                                psk[p][:], wk_sb[:, c, p * 128:(p + 1) * 128], xt[:],
                                start=(c == 0), stop=(c == 7),
                            )
                        for sc in range(4):
                            nc.tensor.matmul(
                                psv[sc][:], xt[:, sc * 128:(sc + 1) * 128], wv_sb[:, c, :],
                                start=(c == 0), stop=(c == 7),
                            )
                    for p in range(4):
                        nc.vector.tensor_copy(kT_t[0][:, p, sb * 512:(sb + 1) * 512], psk[p][:])
                    for sc in range(4):
                        nc.vector.tensor_copy(
                            vE_t[0][:, sb * 4 + sc, :, 0:64], psv[sc][:]
                        )

            # attention-phase PSUM pools (after the prologue pool is closed)
            ps_s = stk.enter_context(tc.tile_pool(name="ps_s", bufs=2, space="PSUM"))
            ps_av = stk.enter_context(tc.tile_pool(name="ps_av", bufs=1, space="PSUM"))
            aux_k = stk.enter_context(tc.tile_pool(name="aux_k", bufs=1, space="PSUM"))
            aux_v = stk.enter_context(tc.tile_pool(name="aux_v", bufs=1, space="PSUM"))

            # --------- deferred kv-projection steps for modality m ----------
            def make_feeder(m, sbs=(0, 1, 2, 3), weights=None):
                f = Feeder()
                if weights is None:
                    new_kv_tiles(m)
                    wk_sb, wv_sb = load_kv_weights(m)
                else:
                    wk_sb, wv_sb = weights
                for sb in sbs:
                    state = {}

                    def load_x(sb=sb, state=state):
                        tiles = []
                        for c in range(8):
                            xt = xts.tile([128, 512], BF16, tag="xt")
                            nc.sync.dma_start(
                                out=xt[:],
                                in_=xkvT[m, c * 128:(c + 1) * 128, sb * 512:(sb + 1) * 512],
                            )
                            tiles.append(xt)
                        state["xt"] = tiles

                    f.add(load_x, True)
                    for p in range(4):
                        chain = {}

                        def k_step(c, p=p, sb=sb, state=state, chain=chain):
                            if c == 0:
                                chain["ps"] = aux_k.tile([128, 512], F32, tag="ak", name="akc")
                            nc.tensor.matmul(
                                chain["ps"][:],
                                wk_sb[:, c, p * 128:(p + 1) * 128],
                                state["xt"][c][:],
                                start=(c == 0), stop=(c == 7),
                            )
                            if c == 7:
                                nc.vector.tensor_copy(
                                    kT_t[m][:, p, sb * 512:(sb + 1) * 512], chain["ps"][:]
                                )

                        for c in range(8):
                            f.add(lambda c=c, k=k_step: k(c), c == 7)
                    for sc in range(4):
                        chain = {}

                        def v_step(c, sc=sc, sb=sb, state=state, chain=chain):
                            if c == 0:
                                chain["ps"] = aux_v.tile([128, 8, 64], F32, tag="av", name="avc")
                            nc.tensor.matmul(
                                chain["ps"][:],
                                state["xt"][c][:, sc * 128:(sc + 1) * 128],
                                wv_sb[:, c, :],
                                start=(c == 0), stop=(c == 7),
                            )
                            if c == 7:
                                nc.vector.tensor_copy(
                                    vE_t[m][:, sb * 4 + sc, :, 0:64], chain["ps"][:]
                                )

                        for c in range(8):
                            f.add(lambda c=c, v=v_step: v(c), c == 7)
                return f

            # ---------------- attention + W1, modality-pipelined -------------
            mf = MultiFeeder()
            mf.append(make_feeder(0, sbs=(1, 2, 3), weights=kv0_w))
            pending_w1 = []

            def issue_w1(m, qt, av_t):
                for jc in range(4):
                    py1 = aux_k.tile([128, 512], F32, tag="ak", name="py1")
                    for hp4 in range(4):
                        nc.tensor.matmul(
                            py1[:], w1_sb[:, hp4, jc, :], av_t[:, hp4, :],
                            start=(hp4 == 0), stop=(hp4 == 3),
                        )
                    if m == 0:
                        nc.vector.tensor_copy(y1acc[:, qt, jc, :], py1[:])
                    else:
                        nc.vector.tensor_tensor(
                            y1acc[:, qt, jc, :], py1[:], y1acc[:, qt, jc, :], ADD
                        )

            # (global-block-deadline, cumulative-steps) for the paced
            # kv-projection stream: sb-chunk j of modality m must be done
            # before block (m*160 + qt_j's first diag use); lead by 8 blocks.
            deadlines = [(12, 65), (48, 130), (100, 195), (152, 260),
                         (172, 325), (208, 390), (260, 455), (312, 520),
                         (332, 585), (368, 650), (420, 715)]

            def fill_target(g):
                prev_g, prev_c = 0, 0
                for dg, dc in deadlines:
                    if g < dg:
                        return prev_c + (g - prev_g) * (dc - prev_c) / (dg - prev_g)
                    prev_g, prev_c = dg, dc
                return 715

            gblk = [0]
            for m in range(M3):
                if m + 1 < M3:
                    mf.append(make_feeder(m + 1))
                kT_m, vE_m = kT_t[m], vE_t[m]
                for qt in range(4):
                    av_t = av_p.tile([128, 4, 512], BF16, tag="avt")
                    for hp in range(4):
                        hA, hB = 2 * hp, 2 * hp + 1
                        pav = ps_av.tile([128, 2, 512], F32, tag="pav")
                        nkt = 4 * (qt + 1)

                        def issue_scores(kt):
                            c0 = max(kt - 4 * qt, 0) * 128
                            psc = ps_s.tile([128, 2, 512], F32, tag="psc")
                            nc.tensor.matmul(
                                psc[:, 0, c0:512],
                                kT_m[0:64, hp, kt * 128:(kt + 1) * 128],
                                qT_sb[0:64, hp, qt * 512 + c0:(qt + 1) * 512],
                                start=True, stop=True,
                            )
                            nc.tensor.matmul(
                                psc[:, 1, c0:512],
                                kT_m[64:128, hp, kt * 128:(kt + 1) * 128],
                                qT_sb[64:128, hp, qt * 512 + c0:(qt + 1) * 512],
                                start=True, stop=True,
                            )
                            return psc

                        # two score pairs in flight before boundary work so
                        # ScalarE has buffered exps across the group seam.
                        pscq = [issue_scores(0)]
                        if nkt > 1:
                            pscq.append(issue_scores(1))
                        if pending_w1:
                            mf.finish_chain()
                            issue_w1(*pending_w1.pop(0))
                        mf.feed(6)
                        for kt in range(nkt):
                            qs = kt - 4 * qt
                            c0 = max(qs, 0) * 128
                            psc_cur = pscq.pop(0)
                            pr = probs_p.tile([128, 2, 512], BF16, tag="pr")
                            nc.scalar.activation(
                                pr[:, :, c0:512], psc_cur[:, :, c0:512], EXP
                            )
                            if qs >= 0:
                                nc.vector.tensor_tensor(
                                    pr[:, 0, c0:c0 + 128], pr[:, 0, c0:c0 + 128],
                                    tri_sb[:], MULT
                                )
                                nc.vector.tensor_tensor(
                                    pr[:, 1, c0:c0 + 128], pr[:, 1, c0:c0 + 128],
                                    tri_sb[:], MULT
                                )
                            if kt + 2 < nkt:
                                pscq.append(issue_scores(kt + 2))
                            nc.tensor.matmul(
                                pav[0:65, 0, c0:512],
                                vE_m[:, kt, hA, :],
                                pr[:, 0, c0:512],
                                start=(kt == 0), stop=(kt == nkt - 1),
                                skip_group_check=True,
                            )
                            nc.tensor.matmul(
                                pav[0:65, 1, c0:512],
                                vE_m[:, kt, hB, :],
                                pr[:, 1, c0:512],
                                start=(kt == 0), stop=(kt == nkt - 1),
                                skip_group_check=True,
                            )
                            gblk[0] += 1
                            need = fill_target(gblk[0]) - mf.fed
                            mf.feed(int(min(max(need, 0), 6)))
                        # denominator drain: copy row 64, reciprocal, broadcast,
                        # then normalize both heads into av_t.
                        dnm = dnm_p.tile([1, 2, 512], F32, tag="dnm")
                        nc.vector.tensor_copy(dnm[0:1, :, :], pav[64:65, :, :])
                        nc.vector.reciprocal_approx_fast(dnm[0:1, :, :], dnm[0:1, :, :])
                        rbc = rbc_p.tile([128, 2, 512], F32, tag="rbc")
                        nc.gpsimd.partition_broadcast(
                            rbc[0:64, :, :], dnm[0:1, :, :], channels=64
                        )
                        nc.vector.tensor_tensor(
                            av_t[0:64, hp, :], pav[0:64, 0, :], rbc[0:64, 0, :], MULT
                        )
                        stg = stg_p.tile([64, 512], BF16, tag="stg")
                        nc.vector.tensor_tensor(
                            stg[0:64, :], pav[0:64, 1, :], rbc[0:64, 1, :], MULT
                        )
                        # head B's rows move to partitions 64-127 so W1 can
                        # contract the full head pair (K=128) in one matmul.
                        nc.sync.dma_start(
                            out=av_t[64:128, hp, :], in_=stg[0:64, :]
                        )
                    pending_w1.append((m, qt, av_t))

            while pending_w1:
                mf.finish_chain()
                issue_w1(*pending_w1.pop(0))
            for qt in range(4):
                for jc in range(4):
                    nc.sync.dma_start(
                        out=y1p[jc, :, qt * 512:(qt + 1) * 512],
                        in_=y1acc[:, qt, jc, :],
                    )
    nc.compile()
    return nc


def _build_launch_b():
    nc = bacc.Bacc("TRN2", target_bir_lowering=False, debug=False, num_devices=N_CORES)
    y1h = nc.dram_tensor("y1h", [128, 4, 1024], F32, kind="ExternalInput").ap()
    b1s = nc.dram_tensor("b1s", [128, 4], F32, kind="ExternalInput").ap()
    w2 = nc.dram_tensor("w2", [128, 4, 1024], BF16, kind="ExternalInput").ap()
    b2 = nc.dram_tensor("b2", [1, 1024], BF16, kind="ExternalInput").ap()
    ob = nc.dram_tensor("ob", [1024, 1024], F32, kind="ExternalOutput").ap()

    with tile.TileContext(nc) as tc:
        with tc.tile_pool(name="sg", bufs=1) as sg, \
             tc.tile_pool(name="ot", bufs=4) as ot_p, \
             tc.tile_pool(name="po", bufs=4, space="PSUM") as po_p:
            y1f = sg.tile([128, 4, 1024], F32, tag="y1f")
            y1t = sg.tile([128, 4, 1024], BF16, tag="y1tt")
            w2_sb = sg.tile([128, 4, 1024], BF16, tag="w2")
            b1_sb = sg.tile([128, 4], F32, tag="b1")
            b2_sb = sg.tile([128, 1024], BF16, tag="b2")
            ones_b = sg.tile([128, 128], BF16, tag="onesb")

            nc.sync.dma_start(out=y1f[:], in_=y1h[:])
            nc.sync.dma_start(out=w2_sb[:], in_=w2[:])
            nc.sync.dma_start(out=b1_sb[:], in_=b1s[:])
            nc.sync.dma_start(out=b2_sb[0:1, :], in_=b2[:])
            nc.vector.memset(ones_b[:], 1.0)

            for jc in range(4):
                nc.scalar.activation(
                    y1t[:, jc, :], y1f[:, jc, :], TANH, bias=b1_sb[:, jc:jc + 1], scale=1.0
                )
            for qc in range(8):
                for ch in range(2):
                    po = po_p.tile([128, 512], F32, tag="po")
                    for jc in range(4):
                        nc.tensor.matmul(
                            po[:], y1t[:, jc, qc * 128:(qc + 1) * 128],
                            w2_sb[:, jc, ch * 512:(ch + 1) * 512],
                            start=(jc == 0), stop=False, skip_group_check=True,
                        )
                    nc.tensor.matmul(
                        po[:], ones_b[0:1, 0:128], b2_sb[0:1, ch * 512:(ch + 1) * 512],
                        start=False, stop=True, skip_group_check=True,
                    )
                    o_t = ot_p.tile([128, 512], F32, tag="ot")
                    nc.vector.tensor_copy(o_t[:], po[:])
                    nc.sync.dma_start(
                        out=ob[qc * 128:(qc + 1) * 128, ch * 512:(ch + 1) * 512], in_=o_t[:]
                    )
    nc.compile()
    return nc


def _pack_wqk(w, scale=None):
    """[8, C, HS] per-head -> pair-packed [8 c-chunk, 128 c-in, 512 (pair*128)]."""
    if scale is not None:
        w = w * scale
    a = w.reshape(4, 2, C, HS).transpose(2, 0, 1, 3).reshape(C, 4, 128)
    return np.ascontiguousarray(a.reshape(8, 128, 512)).astype(BF)


def _pack_wv(w):
    """[8, C, HS] -> [8 c-chunk, 128 c-in, 512 (h*64)]."""
    a = w.transpose(1, 0, 2).reshape(C, 512)
    return np.ascontiguousarray(a.reshape(8, 128, 512)).astype(BF)


def kernel(query_x, kv_x, Wq, Wkv, W1, b1, W2, b2):
    if "a" not in _CACHE:
        _CACHE["a"] = _build_launch_a()
        _CACHE["b"] = _build_launch_b()
    nc_a, nc_b = _CACHE["a"], _CACHE["b"]

    query_x = np.asarray(query_x, dtype=np.float32)
    kv_x = np.asarray(kv_x, dtype=np.float32)
    Wq = np.asarray(Wq, dtype=np.float32)
    Wkv = np.asarray(Wkv, dtype=np.float32)
    W1 = np.asarray(W1, dtype=np.float32)
    b1 = np.asarray(b1, dtype=np.float32)
    W2 = np.asarray(W2, dtype=np.float32)
    b2 = np.asarray(b2, dtype=np.float32)

    tri = np.triu(np.ones((128, 128), dtype=np.float32)).astype(BF)

    # host-side pretranspose (hoisted out of the device kernel)
    xqT_all = [np.ascontiguousarray(query_x[b].T).astype(BF) for b in range(B)]
    xkvT_all = [
        np.ascontiguousarray(kv_x[:, b].transpose(0, 2, 1)).astype(BF) for b in range(B)
    ]

    in_maps = []
    for core in range(N_CORES):
        b, g = core // 2, core % 2
        hs_sl = slice(g * 8, g * 8 + 8)
        w1h = W1[g * 512:(g + 1) * 512]  # [512 hd, 512 j]
        in_maps.append({
            "xqT": xqT_all[b],
            "xkvT": xkvT_all[b],
            "wq": _pack_wqk(Wq[hs_sl], scale=HS ** -0.5),
            "wk": np.stack([_pack_wqk(Wkv[m, hs_sl, :, :HS]) for m in range(M3)]),
            "wv": np.stack([_pack_wv(Wkv[m, hs_sl, :, HS:]) for m in range(M3)]),
            "w1": np.ascontiguousarray(
                w1h.reshape(4, 2, 64, 4, 128).transpose(1, 2, 0, 3, 4)
                .reshape(128, 4, 4, 128)
            ).astype(BF),
            "tri": tri,
        })

    res_a = run_bass_kernel_spmd(nc_a, in_maps, core_ids=list(range(N_CORES)))

    # host glue: add the two head-group partials per batch
    w2p = np.ascontiguousarray(W2.reshape(4, 128, 1024).transpose(1, 0, 2)).astype(BF)
    b1s = np.ascontiguousarray(b1.reshape(4, 128).T)
    b2p = b2.reshape(1, 1024).astype(BF)
    in_maps_b = []
    for core in range(N_CORES):
        b, g = core // 2, core % 2
        y1 = (res_a.results[2 * b]["y1p"].astype(np.float32)
              + res_a.results[2 * b + 1]["y1p"].astype(np.float32))  # [4,128,S]
        y1h = np.ascontiguousarray(y1[:, :, g * 1024:(g + 1) * 1024].transpose(1, 0, 2))
        in_maps_b.append({"y1h": y1h, "b1s": b1s, "w2": w2p, "b2": b2p})

    res_b = run_bass_kernel_spmd(nc_b, in_maps_b, core_ids=list(range(N_CORES)))

    out = np.empty((B, S, C), dtype=np.float32)
    for core in range(N_CORES):
        b, g = core // 2, core % 2
        out[b, g * 1024:(g + 1) * 1024, :] = res_b.results[core]["ob"]
    return out
